# revision 4
# baseline (speedup 1.0000x reference)
"""Trainium2 Bass kernel for a single-step attention-GRU decoder
(embedding lookup + Bahdanau attention + GRU cell + vocab projection
with log_softmax), tensor-parallel over 8 NeuronCores.

Sharding: the vocab projection (out_w, the dominant 412MB load) and the
comb/GRU matvecs are sharded 8 ways; the tiny attention state is
replicated. Two 1KB AllGathers stitch the sharded activations back
together (x after comb+relu, h_new after the GRU cell), and a 4-byte
AllGather combines the per-shard softmax normalizers.

Host side only reshapes/slices inputs (one embedding row gather, weight
transposes into SBUF-friendly layouts) and concatenates the per-core
outputs.
"""
import sys

if "/opt/trn_rl_repo" not in sys.path:
    sys.path.insert(0, "/opt/trn_rl_repo")

import numpy as np

import concourse.bass as bass
import concourse.mybir as mybir
import concourse.tile as tile
from concourse.tile import ScopedClock
from concourse.bass_utils import run_bass_kernel_spmd

# ---------------------------------------------------------------- constants
H = 2048
V = 50257
M = 64
NC = 8

HPC = H // NC            # 256, per-core shard of H
GPC = 3 * HPC            # 768, per-core shard of the 3 GRU gates
VPC = 6283               # per-core vocab shard
VPAD = VPC * NC          # 50264
NBLK = 13                # vocab blocks per core: 12 x 512 + 1 x 139
BLK = 512
LBLK = VPC - 12 * BLK    # 139
KCH = H // 128           # 16 contraction chunks of 128

F32 = mybir.dt.float32
WDT = mybir.dt.bfloat16  # dtype for the streamed weights / matvec operands
NEG_BIG = -1e30

if WDT == mybir.dt.bfloat16:
    import ml_dtypes
    NP_WDT = ml_dtypes.bfloat16
else:
    NP_WDT = np.float32


# ------------------------------------------------------- sync-wait workarounds
# This walrus build rejects instructions carrying more than 1 sync wait.
MAX_WAITS = 1


def _split_excess_waits(nc):
    """Move excess sem waits of any instruction onto freshly inserted
    same-engine nops placed immediately before it (engines execute their
    stream in order, so the waits still gate the instruction)."""
    for f in nc.m.functions:
        for bb in f.blocks:
            lst = bb.instructions
            i = 0
            while i < len(lst):
                ins = lst[i]
                si = ins.sync_info
                waits = list(si.on_wait) if si is not None and si.on_wait else []
                if len(waits) > MAX_WAITS:
                    keep = waits[-MAX_WAITS:]
                    extra = waits[:-MAX_WAITS]
                    updates = list(si.on_update) if si.on_update else []
                    ins.sync_info = mybir.SyncInfo(on_wait=keep, on_update=updates)
                    eng = nc.engines[ins.engine]
                    pos = i
                    for j in range(0, len(extra), MAX_WAITS):
                        chunk = extra[j : j + MAX_WAITS]
                        nop = eng.nop(nofuse=True, hint="waitsplit").ins
                        # nop was appended to the current bb; relocate it
                        cur = nc.cur_bb.bb.instructions
                        assert cur[-1] is nop
                        cur.pop()
                        nop.sync_info = mybir.SyncInfo(on_wait=chunk, on_update=[])
                        lst.insert(pos, nop)
                        pos += 1
                        i += 1
                i += 1


def _drain_and_barrier_split(self, tick_clock, wait_clock):
    nc = self.nc
    probe = nc.sync.nop(nofuse=True, hint="drain_wait_probe")
    wait_clock.add_sem_waits(probe.ins, ScopedClock({None: tick_clock.global_clock}))
    si = probe.ins.sync_info
    waits = list(si.on_wait) if si is not None else []
    probe.ins.sync_info = None
    sems = {h.num: h for h in self.sems.allocated().values()}
    for w in waits:
        h = sems.get(w.id)
        assert h is not None, f"no handle for sem {w.id} ({w.ant_name})"
        assert w.wait_mode == "sem-ge-imm", w.wait_mode
        nc.sync.wait_ge(h, w.wait_value)
    nc.sync.drain()
    nc.all_engine_barrier()
    assert self.sems is not None
    popped = nc._tile_sem_poison_stack.pop()
    assert popped is self._sem_poison
    nc.clear_and_free_semaphores(list(self.sems.allocated().values()))
    nc.all_engine_barrier()
    _split_excess_waits(nc)


tile.TileContext._drain_and_barrier = _drain_and_barrier_split


# ---------------------------------------------------------------- device code
def _build_nc():
    nc = bass.Bass()
    d = {}

    def din(name, shape, dt=F32):
        d[name] = nc.dram_tensor(name, shape, dt, kind="ExternalInput")
        return d[name]

    def dout(name, shape, dt=F32):
        d[name] = nc.dram_tensor(name, shape, dt, kind="ExternalOutput")
        return d[name]

    # small replicated state
    din("embT", [128, KCH], WDT)
    din("h0T", [128, KCH], WDT)
    din("h0s", [1, HPC])
    din("attn_b_t", [1, M])
    din("comb_b_t", [1, HPC])
    din("bih_t", [1, GPC])
    din("bhh_t", [1, GPC])
    din("outb_t", [1, VPC])
    din("ones11", [1, 1])
    din("id16", [KCH, KCH])
    # weights (host pre-laid-out for dense [128, X] DMA)
    din("enc_s", [M, H], WDT)
    din("attn_w_s", [128, 32 * M], WDT)
    din("comb_w_s", [128, 32 * HPC], WDT)
    din("wih_s", [128, KCH * GPC], WDT)
    din("whh_s", [128, KCH * GPC], WDT)
    din("outw_s", [128, KCH * VPC], WDT)
    # outputs
    dout("lp", [1, VPC])
    dout("hnew_ag", [NC, HPC])
    dout("aw_out", [1, M])

    rg = [list(range(NC))]

    with tile.TileContext(nc) as tc:
        with (
            tc.tile_pool(name="const", bufs=1) as cpool,
            tc.tile_pool(name="wstream", bufs=2) as wpool,
            tc.tile_pool(name="panel", bufs=3) as ppool,
            tc.tile_pool(name="vec", bufs=1) as vpool,
            tc.tile_pool(name="psum", bufs=1, space="PSUM") as qpool,
            tc.tile_pool(name="dram", bufs=1, space="DRAM") as dpool,
        ):
            # ---- small tile loads
            embT = cpool.tile([128, KCH], WDT)
            nc.sync.dma_start(embT[:], d["embT"][:])
            h0T = cpool.tile([128, KCH], WDT)
            nc.sync.dma_start(h0T[:], d["h0T"][:])
            h0s = cpool.tile([1, HPC], F32)
            nc.sync.dma_start(h0s[:], d["h0s"][:])
            attn_b = cpool.tile([1, M], F32)
            nc.sync.dma_start(attn_b[:], d["attn_b_t"][:])
            comb_b = cpool.tile([1, HPC], F32)
            nc.sync.dma_start(comb_b[:], d["comb_b_t"][:])
            bih = cpool.tile([1, GPC], F32)
            nc.sync.dma_start(bih[:], d["bih_t"][:])
            bhh = cpool.tile([1, GPC], F32)
            nc.sync.dma_start(bhh[:], d["bhh_t"][:])
            ones11 = cpool.tile([1, 1], F32)
            nc.sync.dma_start(ones11[:], d["ones11"][:])
            id16 = cpool.tile([KCH, KCH], F32)
            nc.sync.dma_start(id16[:], d["id16"][:])
            enc_t = cpool.tile([M, H], WDT)
            nc.sync.dma_start(enc_t[:], d["enc_s"][:])
            attnw_t = cpool.tile([128, 32 * M], WDT)
            nc.sync.dma_start(attnw_t[:], d["attn_w_s"][:])

            # collective bounce buffers
            agx_in = dpool.tile([1, HPC], F32)
            agx_out = dpool.tile([NC, HPC], F32)
            agh_in = dpool.tile([1, HPC], F32)
            agh_out = dpool.tile([NC, HPC], F32)
            ags_in = dpool.tile([1, 1], F32)
            ags_out = dpool.tile([NC, 1], F32)

            # ---- attention: logits = [emb, h0] @ attn_w.T + attn_b
            ps_attn = qpool.tile([1, M], F32, tag="pA", bufs=2)
            for k in range(32):
                lhsT = embT[:, k : k + 1] if k < 16 else h0T[:, k - 16 : k - 15]
                nc.tensor.matmul(
                    ps_attn[:],
                    lhsT,
                    attnw_t[:, k * M : (k + 1) * M],
                    start=(k == 0),
                    stop=(k == 31),
                )
            aw_sb = vpool.tile([1, M], F32)
            nc.vector.tensor_add(aw_sb[:], ps_attn[:], attn_b[:])
            mx = vpool.tile([1, 1], F32)
            nc.vector.reduce_max(mx[:], aw_sb[:], axis=mybir.AxisListType.X)
            mxn = vpool.tile([1, 1], F32)
            nc.vector.tensor_scalar_mul(mxn[:], mx[:], -1.0)
            expv = vpool.tile([1, M], F32)
            sm = vpool.tile([1, 1], F32)
            nc.scalar.activation(
                expv[:], aw_sb[:], mybir.ActivationFunctionType.Exp,
                bias=mxn[:], scale=1.0, accum_out=sm[:],
            )
            rs = vpool.tile([1, 1], F32)
            nc.vector.reciprocal(rs[:], sm[:])
            awn = vpool.tile([1, M], F32)
            nc.vector.tensor_scalar_mul(awn[:], expv[:], rs[:])
            nc.sync.dma_start(d["aw_out"][:], awn[:])

            # transpose attn weights row -> column [M, 1]
            ps_awT = qpool.tile([M, 1], F32, tag="pB", bufs=2)
            nc.tensor.matmul(ps_awT[:], awn[:], ones11[:], start=True, stop=True)
            awT = vpool.tile([M, 1], WDT)
            nc.scalar.copy(awT[:], ps_awT[:])

            # attn_applied = attn_weights @ encoder_outputs, partition-major
            ps_app = qpool.tile([128, KCH], F32, tag="pB", bufs=2)
            for j in range(KCH):
                nc.tensor.matmul(
                    ps_app[:, j : j + 1],
                    enc_t[:, j * 128 : (j + 1) * 128],
                    awT[:],
                    start=True,
                    stop=True,
                )
            appT = vpool.tile([128, KCH], WDT)
            nc.scalar.copy(appT[:], ps_app[:])

            # ---- comb: x_shard = relu([emb, attn_applied] @ comb_w.T + b)
            ps_x = qpool.tile([1, HPC], F32, tag="pA", bufs=2)
            for g in range(4):  # stream comb weights in 4 chunks of 8 k's
                cw = wpool.tile([128, 8 * HPC], WDT, tag="cw")
                nc.sync.dma_start(
                    cw[:], d["comb_w_s"][:, g * 8 * HPC : (g + 1) * 8 * HPC]
                )
                for kk in range(8):
                    k = g * 8 + kk
                    lhsT = embT[:, k : k + 1] if k < 16 else appT[:, k - 16 : k - 15]
                    nc.tensor.matmul(
                        ps_x[:],
                        lhsT,
                        cw[:, kk * HPC : (kk + 1) * HPC],
                        start=(k == 0),
                        stop=(k == 31),
                    )
            xsh = vpool.tile([1, HPC], F32)
            nc.vector.tensor_add(xsh[:], ps_x[:], comb_b[:])
            nc.scalar.activation(xsh[:], xsh[:], mybir.ActivationFunctionType.Relu)

            # AllGather x
            nc.sync.dma_start(agx_in[:], xsh[:])
            nc.gpsimd.collective_compute(
                "AllGather", mybir.AluOpType.bypass, replica_groups=rg,
                ins=[agx_in.opt()], outs=[agx_out.opt()],
            )
            x16 = vpool.tile([KCH, 128], F32)
            nc.sync.dma_start(x16[:], agx_out.rearrange("a (b f) -> (a b) f", f=128))
            ps_xT = qpool.tile([128, KCH], F32, tag="pB", bufs=2)
            nc.tensor.transpose(ps_xT[:], x16[:], id16[:])
            xT = vpool.tile([128, KCH], WDT)
            nc.scalar.copy(xT[:], ps_xT[:])

            # ---- GRU gates: gi = x @ w_ih_shard.T ; gh = h0 @ w_hh_shard.T
            ps_gia = qpool.tile([1, BLK], F32, tag="pA", bufs=2)
            ps_gib = qpool.tile([1, GPC - BLK], F32, tag="pB", bufs=2)
            for g in range(4):  # stream w_ih in 4 chunks of 4 k's
                wi = wpool.tile([128, 4 * GPC], WDT, tag="wi")
                nc.sync.dma_start(
                    wi[:], d["wih_s"][:, g * 4 * GPC : (g + 1) * 4 * GPC]
                )
                for kk in range(4):
                    k = g * 4 + kk
                    nc.tensor.matmul(
                        ps_gia[:], xT[:, k : k + 1],
                        wi[:, kk * GPC : kk * GPC + BLK],
                        start=(k == 0), stop=(k == 15),
                    )
                    nc.tensor.matmul(
                        ps_gib[:], xT[:, k : k + 1],
                        wi[:, kk * GPC + BLK : (kk + 1) * GPC],
                        start=(k == 0), stop=(k == 15),
                    )
            ps_gha = qpool.tile([1, BLK], F32, tag="pA", bufs=2)
            ps_ghb = qpool.tile([1, GPC - BLK], F32, tag="pB", bufs=2)
            for g in range(4):
                wh = wpool.tile([128, 4 * GPC], WDT, tag="wh")
                nc.sync.dma_start(
                    wh[:], d["whh_s"][:, g * 4 * GPC : (g + 1) * 4 * GPC]
                )
                for kk in range(4):
                    k = g * 4 + kk
                    nc.tensor.matmul(
                        ps_gha[:], h0T[:, k : k + 1],
                        wh[:, kk * GPC : kk * GPC + BLK],
                        start=(k == 0), stop=(k == 15),
                    )
                    nc.tensor.matmul(
                        ps_ghb[:], h0T[:, k : k + 1],
                        wh[:, kk * GPC + BLK : (kk + 1) * GPC],
                        start=(k == 0), stop=(k == 15),
                    )
            gi = vpool.tile([1, GPC], F32)
            nc.vector.tensor_add(gi[:, :BLK], ps_gia[:], bih[:, :BLK])
            nc.vector.tensor_add(gi[:, BLK:GPC], ps_gib[:], bih[:, BLK:GPC])
            gh = vpool.tile([1, GPC], F32)
            nc.vector.tensor_add(gh[:, :BLK], ps_gha[:], bhh[:, :BLK])
            nc.vector.tensor_add(gh[:, BLK:GPC], ps_ghb[:], bhh[:, BLK:GPC])

            # gates r,z via sigmoid(x) = 1/(1+exp(-x)) (keeps one ACT table set)
            trz = vpool.tile([1, 2 * HPC], F32)
            nc.vector.tensor_add(trz[:], gi[:, : 2 * HPC], gh[:, : 2 * HPC])
            erz = vpool.tile([1, 2 * HPC], F32)
            nc.scalar.activation(
                erz[:], trz[:], mybir.ActivationFunctionType.Exp, scale=-1.0
            )
            drz = vpool.tile([1, 2 * HPC], F32)
            nc.vector.tensor_scalar_add(drz[:], erz[:], 1.0)
            rz = vpool.tile([1, 2 * HPC], F32)
            nc.vector.reciprocal(rz[:], drz[:])
            # n = tanh(i_n + r*h_n);  tanh(t) = 2/(1+exp(-2t)) - 1
            t1 = vpool.tile([1, HPC], F32)
            nc.vector.tensor_mul(t1[:], rz[:, :HPC], gh[:, 2 * HPC : GPC])
            t2 = vpool.tile([1, HPC], F32)
            nc.vector.tensor_add(t2[:], gi[:, 2 * HPC : GPC], t1[:])
            e2 = vpool.tile([1, HPC], F32)
            nc.scalar.activation(
                e2[:], t2[:], mybir.ActivationFunctionType.Exp, scale=-2.0
            )
            d2 = vpool.tile([1, HPC], F32)
            nc.vector.tensor_scalar_add(d2[:], e2[:], 1.0)
            s2 = vpool.tile([1, HPC], F32)
            nc.vector.reciprocal(s2[:], d2[:])
            n_sb = vpool.tile([1, HPC], F32)
            nc.scalar.activation(
                n_sb[:], s2[:], mybir.ActivationFunctionType.Copy,
                bias=-1.0, scale=2.0,
            )
            # h_new = n + z*(h0 - n)
            t3 = vpool.tile([1, HPC], F32)
            nc.vector.tensor_sub(t3[:], h0s[:], n_sb[:])
            t4 = vpool.tile([1, HPC], F32)
            nc.vector.tensor_mul(t4[:], rz[:, HPC : 2 * HPC], t3[:])
            hns = vpool.tile([1, HPC], F32)
            nc.vector.tensor_add(hns[:], n_sb[:], t4[:])

            # AllGather h_new
            nc.sync.dma_start(agh_in[:], hns[:])
            nc.gpsimd.collective_compute(
                "AllGather", mybir.AluOpType.bypass, replica_groups=rg,
                ins=[agh_in.opt()], outs=[agh_out.opt()],
            )
            nc.sync.dma_start(d["hnew_ag"][:], agh_out[:])
            h16 = vpool.tile([KCH, 128], F32)
            nc.sync.dma_start(h16[:], agh_out.rearrange("a (b f) -> (a b) f", f=128))
            ps_hT = qpool.tile([128, KCH], F32, tag="pB", bufs=2)
            nc.tensor.transpose(ps_hT[:], h16[:], id16[:])
            hT = vpool.tile([128, KCH], WDT)
            nc.scalar.copy(hT[:], ps_hT[:])

            # ---- vocab projection + shard-local softmax stats
            logits = cpool.tile([1, VPC], F32)
            accu = cpool.tile([1, 16], F32)
            for nb in range(NBLK):
                nv = BLK if nb < 12 else LBLK
                off = nb * KCH * BLK
                panel = ppool.tile([128, KCH * BLK], WDT, tag="panel")
                nc.sync.dma_start(
                    panel[:, : KCH * nv], d["outw_s"][:, off : off + KCH * nv]
                )
                ps_l = qpool.tile([1, BLK], F32, tag="pL", bufs=3)
                for k in range(KCH):
                    nc.tensor.matmul(
                        ps_l[:, :nv], hT[:, k : k + 1],
                        panel[:, k * nv : (k + 1) * nv],
                        start=(k == 0), stop=(k == 15),
                    )
                ob = wpool.tile([1, BLK], F32, tag="ob")
                nc.sync.dma_start(
                    ob[:, :nv], d["outb_t"][:, nb * BLK : nb * BLK + nv]
                )
                lsl = logits[:, nb * BLK : nb * BLK + nv]
                nc.vector.tensor_add(lsl, ps_l[:, :nv], ob[:, :nv])
                ex = wpool.tile([1, BLK], F32, tag="ex")
                nc.scalar.activation(
                    ex[:, :nv], lsl, mybir.ActivationFunctionType.Exp,
                    accum_out=accu[:, nb : nb + 1],
                )
            stot = vpool.tile([1, 1], F32)
            nc.vector.reduce_sum(stot[:], accu[:, :NBLK], axis=mybir.AxisListType.X)
            nc.sync.dma_start(ags_in[:], stot[:])
            nc.gpsimd.collective_compute(
                "AllGather", mybir.AluOpType.bypass, replica_groups=rg,
                ins=[ags_in.opt()], outs=[ags_out.opt()],
            )
            s8 = vpool.tile([1, NC], F32)
            nc.sync.dma_start(s8[:], ags_out.rearrange("a b -> b a"))
            S = vpool.tile([1, 1], F32)
            nc.vector.reduce_sum(S[:], s8[:], axis=mybir.AxisListType.X)
            lse = vpool.tile([1, 1], F32)
            nc.scalar.activation(lse[:], S[:], mybir.ActivationFunctionType.Ln)
            nlse = vpool.tile([1, 1], F32)
            nc.vector.tensor_scalar_mul(nlse[:], lse[:], -1.0)
            nc.vector.tensor_scalar_add(logits[:], logits[:], nlse[:])
            nc.sync.dma_start(d["lp"][:], logits[:])

    return nc


# ------------------------------------------------------------------ host prep
def _chunked(wT, n_out):
    """[K, n_out] (K=k*128+p) -> [128, (K/128)*n_out] with layout
    arr[p, k*n_out + n] = wT[k*128+p, n]."""
    K = wT.shape[0]
    kch = K // 128
    return np.ascontiguousarray(
        wT.reshape(kch, 128, n_out).transpose(1, 0, 2).reshape(128, kch * n_out)
    )


def _prep_in_maps(inputs):
    f32 = np.float32
    tok = int(np.asarray(inputs["input_tok"]).reshape(-1)[0])
    emb_row = np.asarray(inputs["emb"][tok], dtype=f32).reshape(H)
    h0 = np.asarray(inputs["hidden"], dtype=f32).reshape(H)
    attn_w = np.asarray(inputs["attn_w"], dtype=f32)
    attn_b = np.asarray(inputs["attn_b"], dtype=f32)
    enc = np.asarray(inputs["encoder_outputs"], dtype=f32)
    comb_w = np.asarray(inputs["comb_w"], dtype=f32)
    comb_b = np.asarray(inputs["comb_b"], dtype=f32)
    w_ih = np.asarray(inputs["w_ih"], dtype=f32)
    w_hh = np.asarray(inputs["w_hh"], dtype=f32)
    b_ih = np.asarray(inputs["b_ih"], dtype=f32)
    b_hh = np.asarray(inputs["b_hh"], dtype=f32)
    out_w = np.asarray(inputs["out_w"], dtype=f32)
    out_b = np.asarray(inputs["out_b"], dtype=f32)

    embT = np.ascontiguousarray(emb_row.reshape(KCH, 128).T).astype(NP_WDT)
    h0T = np.ascontiguousarray(h0.reshape(KCH, 128).T).astype(NP_WDT)
    attn_w_s = _chunked(attn_w.T, M).astype(NP_WDT)          # [128, 32*64]
    enc_s = enc.astype(NP_WDT)
    ones11 = np.ones((1, 1), f32)
    id16 = np.eye(KCH, dtype=f32)

    out_b_pad = np.concatenate([out_b, np.full(VPAD - V, NEG_BIG, f32)])

    shared = {
        "embT": embT, "h0T": h0T,
        "attn_b_t": attn_b.reshape(1, M),
        "ones11": ones11, "id16": id16,
        "enc_s": enc_s, "attn_w_s": attn_w_s,
    }

    in_maps = []
    for c in range(NC):
        sl = slice(c * HPC, (c + 1) * HPC)
        gate_rows = np.concatenate(
            [np.arange(g * H + c * HPC, g * H + (c + 1) * HPC) for g in range(3)]
        )
        comb_w_s = _chunked(
            np.ascontiguousarray(comb_w[sl].T), HPC
        ).astype(NP_WDT)                                      # [128, 32*256]
        wih_s = _chunked(
            np.ascontiguousarray(w_ih[gate_rows].T), GPC
        ).astype(NP_WDT)                                      # [128, 16*768]
        whh_s = _chunked(
            np.ascontiguousarray(w_hh[gate_rows].T), GPC
        ).astype(NP_WDT)
        # out_w shard -> [128, nb*(16*512) + k*512 + n] block-major panels
        lo = c * VPC
        Wc = out_w[lo : min(lo + VPC, V)]
        if Wc.shape[0] < VPC:
            Wc = np.concatenate([Wc, np.zeros((VPC - Wc.shape[0], H), f32)])
        main = np.ascontiguousarray(
            Wc[: 12 * BLK].reshape(12, BLK, KCH, 128)
            .transpose(3, 0, 2, 1).reshape(128, 12 * KCH * BLK)
        )
        tail = np.ascontiguousarray(
            Wc[12 * BLK :].reshape(LBLK, KCH, 128)
            .transpose(2, 1, 0).reshape(128, KCH * LBLK)
        )
        outw_s = np.concatenate([main, tail], axis=1).astype(NP_WDT)

        in_maps.append(dict(
            shared,
            h0s=h0[sl].reshape(1, HPC).copy(),
            comb_b_t=comb_b[sl].reshape(1, HPC).copy(),
            bih_t=b_ih[gate_rows].reshape(1, GPC).copy(),
            bhh_t=b_hh[gate_rows].reshape(1, GPC).copy(),
            outb_t=out_b_pad[lo : lo + VPC].reshape(1, VPC).copy(),
            comb_w_s=comb_w_s, wih_s=wih_s, whh_s=whh_s, outw_s=outw_s,
        ))
    return in_maps


_NC_CACHE = {}
LAST_RESULT = None  # BassKernelResults of the most recent run (for test harness)


def kernel(_profile=False, _tmpdir=None, **inputs):
    global LAST_RESULT
    if "nc" not in _NC_CACHE:
        _NC_CACHE["nc"] = _build_nc()
    nc = _NC_CACHE["nc"]
    in_maps = _prep_in_maps(inputs)
    res = run_bass_kernel_spmd(
        nc, in_maps, core_ids=list(range(NC)),
        trace=_profile, tmpdir=_tmpdir,
    )
    LAST_RESULT = res
    outs = res.results
    lp = np.concatenate([outs[c]["lp"] for c in range(NC)], axis=1)[:, :V]
    h_new = outs[0]["hnew_ag"].reshape(1, 1, H)
    aw = outs[0]["aw_out"].reshape(1, M)
    return (np.ascontiguousarray(lp), h_new, aw)
